# revision 32
# baseline (speedup 1.0000x reference)
"""AvU loss (accuracy-vs-uncertainty) Trainium2 kernel.

Strategy (data parallel over 8 NeuronCores):
  Each sample contributes w = q*r to the denominator and w*[a==u] to the
  numerator, where
     q = c if accurate else (1-c),        c = probs[:,1]
     r = (1-t) if certain else t,         t = tanh(unc)
     a = [label == argmax(probs)],        u = [unc <= unc_th]
  With sign encodings S_a = 2a-1, S_u = 2u-1 (both +-1):
     WS2 := (S_a + c2) * (u01 - t)  where c2 = 2c-1, u01 = [unc<=th]
          = 2 * w * S_a * S_u
  so   sum(w)        = sum(|WS2|) / 2
       sum(w*[a==u]) = (sum(|WS2|) + sum(WS2)) / 4
  Each core computes per-partition partial sums of WS2 (fused into the
  product op via scalar_tensor_tensor accum_out) and of |WS2| (fused into
  the ScalarE Abs activation via accum_out); the host combines the
  8 * 128 * T partials in float64 and finishes the log.
"""

import numpy as np

_N = 16777216
_NCORES = 8
_P = 128
_NC = _N // _NCORES
_E = _NC // _P  # 16384 elements per partition per core
# 8 x 2048 is the measured sweet spot (4 x 4096: -10 us granularity loss;
# 11 mixed tiles: -5 us per-op overhead loss). The last tile is split in two:
# after the final DMA lands, the remaining serial ACT->DVE->ACT chain is the
# only un-overlapped compute, and halving the last tile halves that drain.
_TILES = [2048] * 7 + [1024, 512, 512]
assert sum(_TILES) == _E

_built = {}


def _build(unc_th: float, tiles=None):
    import concourse.bacc as bacc
    import concourse.mybir as mybir
    import concourse.tile as tile

    f32 = mybir.dt.float32
    bf16 = mybir.dt.bfloat16
    i32 = mybir.dt.int32
    Alu = mybir.AluOpType
    Act = mybir.ActivationFunctionType

    tiles = list(_TILES) if tiles is None else list(tiles)
    E = sum(tiles)
    T = len(tiles)

    nc = bacc.Bacc("TRN2")
    probs = nc.dram_tensor("probs", [2 * _P * E], f32, kind="ExternalInput")
    labs = nc.dram_tensor("labs", [_P * E], i32, kind="ExternalInput")
    unc = nc.dram_tensor("unc", [_P * E], f32, kind="ExternalInput")
    out = nc.dram_tensor("out", [_P, 2 * T], f32, kind="ExternalOutput")

    with tile.TileContext(nc) as tc:
        with (
            tc.tile_pool(name="io", bufs=4) as io,
            tc.tile_pool(name="mid", bufs=2) as mid,
            tc.tile_pool(name="acc", bufs=1) as accp,
        ):
            accA = accp.tile([_P, T], f32)  # per-tile per-partition sum(WS2)
            absA = accp.tile([_P, T], f32)  # per-tile per-partition sum(|WS2|)
            neg1 = accp.tile([_P, 1], f32)  # bias vector for Sign activation
            nc.vector.memset(neg1, -1.0)
            base = 0
            for i, F in enumerate(tiles):
                pr_ap = probs[2 * _P * base : 2 * _P * (base + F)].rearrange(
                    "(p f) -> p f", p=_P
                )
                lb_ap = labs[_P * base : _P * (base + F)].rearrange(
                    "(p f) -> p f", p=_P
                )
                un_ap = unc[_P * base : _P * (base + F)].rearrange(
                    "(p f) -> p f", p=_P
                )
                base += F
                pt = io.tile([_P, 2 * F], f32, tag="probs")
                nc.sync.dma_start(out=pt, in_=pr_ap)
                lt = io.tile([_P, F], i32, tag="labs")
                nc.sync.dma_start(out=lt, in_=lb_ap)
                ut = io.tile([_P, F], f32, tag="unc")
                nc.sync.dma_start(out=ut, in_=un_ap)

                p1 = pt[:, 1::2]  # confidences, strided view of interleaved probs

                # tt tile: tanh(unc), later overwritten in place by hm
                tt = mid.tile([_P, F], bf16, tag="tt")
                nc.scalar.activation(tt, ut, Act.Tanh)
                # c2 tile: 2*p1-1, later overwritten by g, ws, aw in place
                c2 = mid.tile([_P, F], bf16, tag="c2")
                nc.scalar.activation(c2, p1, Act.Copy, bias=-1.0, scale=2.0)
                sg = mid.tile([_P, F], bf16, tag="sg")
                nc.scalar.activation(sg, p1, Act.Sign, bias=neg1, scale=2.0)
                # l2 tile: 2*lab-1, overwritten in place by sa
                l2 = mid.tile([_P, F], bf16, tag="l2")
                nc.vector.tensor_scalar(
                    out=l2, in0=lt, scalar1=2.0, scalar2=-1.0,
                    op0=Alu.mult, op1=Alu.add,
                )
                # hm = [unc <= th] - tanh(unc), in place over tt
                nc.vector.scalar_tensor_tensor(
                    tt, ut, float(unc_th), tt, op0=Alu.is_le, op1=Alu.subtract
                )
                # sa = l2 * sg  (= S_a), in place over l2
                nc.vector.tensor_mul(l2, l2, sg)
                # g = sa + c2, in place over c2
                nc.vector.tensor_add(c2, l2, c2)
                # ws = g * hm, in place over c2; fused per-partition sum
                nc.vector.scalar_tensor_tensor(
                    c2, c2, 0.0, tt, op0=Alu.bypass, op1=Alu.mult,
                    accum_out=accA[:, i : i + 1],
                )
                # |ws| on ScalarE, in place; fused per-partition sum
                nc.scalar.activation(
                    c2, c2, Act.Abs, accum_out=absA[:, i : i + 1]
                )
            nc.sync.dma_start(out=out[:, 0:T], in_=accA)
            nc.sync.dma_start(out=out[:, T : 2 * T], in_=absA)
    nc.finalize()  # Bacc: run wait-splitting + register allocation passes
    return nc


def _prep(probs, labels, unc, unc_th):
    probs = np.ascontiguousarray(np.asarray(probs), dtype=np.float32)
    unc = np.ascontiguousarray(np.asarray(unc), dtype=np.float32)
    labels = np.asarray(labels)
    if labels.dtype != np.int32:
        labels = labels.astype(np.int32)  # values are 0/1; lossless narrowing
    labels = np.ascontiguousarray(labels)
    th = float(np.asarray(unc_th))
    assert probs.shape == (_N, 2), probs.shape
    assert unc.shape == (_N,), unc.shape
    assert labels.shape == (_N,), labels.shape

    if th not in _built:
        _built[th] = _build(th)
    nc = _built[th]

    pr = probs.reshape(_NCORES, 2 * _NC)
    lb = labels.reshape(_NCORES, _NC)
    un = unc.reshape(_NCORES, _NC)
    in_maps = [
        {"probs": pr[c], "labs": lb[c], "unc": un[c]} for c in range(_NCORES)
    ]
    return nc, in_maps


def _finish(results):
    S_ws = 0.0
    S_abs = 0.0
    for r in results:
        o = r["out"].astype(np.float64)
        half = o.shape[1] // 2
        S_ws += o[:, :half].sum()
        S_abs += o[:, half:].sum()
    den = S_abs / 2.0
    num = (S_abs + S_ws) / 4.0
    avu = num / (den + 1e-10)
    loss = -1.0 * np.log(avu + 1e-10)
    return np.asarray([loss], dtype=np.float32)


def _run(probs, labels, unc, unc_th, trace=False, **kwargs):
    from concourse.bass_utils import run_bass_kernel_spmd

    nc, in_maps = _prep(probs, labels, unc, unc_th)
    res = run_bass_kernel_spmd(
        nc, in_maps, core_ids=list(range(_NCORES)), trace=trace, **kwargs
    )
    return _finish(res.results), res


def kernel(probs, labels, unc, unc_th):
    out, _ = _run(probs, labels, unc, unc_th, trace=False)
    return out


# revision 33
# speedup vs baseline: 1.0663x; 1.0663x over previous
"""AvU loss (accuracy-vs-uncertainty) Trainium2 kernel.

Strategy (data parallel over 8 NeuronCores):
  Each sample contributes w = q*r to the denominator and w*[a==u] to the
  numerator, where
     q = c if accurate else (1-c),        c = probs[:,1]
     r = (1-t) if certain else t,         t = tanh(unc)
     a = [label == argmax(probs)],        u = [unc <= unc_th]
  With sign encodings S_a = 2a-1, S_u = 2u-1 (both +-1):
     WS2 := (S_a + c2) * (u01 - t)  where c2 = 2c-1, u01 = [unc<=th]
          = 2 * w * S_a * S_u
  so   sum(w)        = sum(|WS2|) / 2
       sum(w*[a==u]) = (sum(|WS2|) + sum(WS2)) / 4
  Each core computes per-partition partial sums of WS2 (fused into the
  product op via scalar_tensor_tensor accum_out) and of |WS2| (fused into
  the ScalarE Abs activation via accum_out); the host combines the
  8 * 128 * T partials in float64 and finishes the log.
"""

import numpy as np

_N = 16777216
_NCORES = 8
_P = 128
_NC = _N // _NCORES
_E = _NC // _P  # 16384 elements per partition per core
# 8 x 2048 is the measured sweet spot (4 x 4096: -10 us granularity loss;
# 11 mixed tiles: -5 us per-op overhead loss). The last tile is split in two:
# after the final DMA lands, the remaining serial ACT->DVE->ACT chain is the
# only un-overlapped compute, and halving the last tile halves that drain.
_TILES = [2048] * 7 + [1024, 1024]
assert sum(_TILES) == _E

_built = {}


def _build(unc_th: float, tiles=None):
    import concourse.bacc as bacc
    import concourse.mybir as mybir
    import concourse.tile as tile

    f32 = mybir.dt.float32
    bf16 = mybir.dt.bfloat16
    i32 = mybir.dt.int32
    Alu = mybir.AluOpType
    Act = mybir.ActivationFunctionType

    tiles = list(_TILES) if tiles is None else list(tiles)
    E = sum(tiles)
    T = len(tiles)

    nc = bacc.Bacc("TRN2")
    probs = nc.dram_tensor("probs", [2 * _P * E], f32, kind="ExternalInput")
    labs = nc.dram_tensor("labs", [_P * E], i32, kind="ExternalInput")
    unc = nc.dram_tensor("unc", [_P * E], f32, kind="ExternalInput")
    out = nc.dram_tensor("out", [_P, 2 * T], f32, kind="ExternalOutput")

    with tile.TileContext(nc) as tc:
        with (
            tc.tile_pool(name="io", bufs=4) as io,
            tc.tile_pool(name="mid", bufs=2) as mid,
            tc.tile_pool(name="acc", bufs=1) as accp,
        ):
            accA = accp.tile([_P, T], f32)  # per-tile per-partition sum(WS2)
            absA = accp.tile([_P, T], f32)  # per-tile per-partition sum(|WS2|)
            neg1 = accp.tile([_P, 1], f32)  # bias vector for Sign activation
            nc.vector.memset(neg1, -1.0)
            base = 0
            for i, F in enumerate(tiles):
                pr_ap = probs[2 * _P * base : 2 * _P * (base + F)].rearrange(
                    "(p f) -> p f", p=_P
                )
                lb_ap = labs[_P * base : _P * (base + F)].rearrange(
                    "(p f) -> p f", p=_P
                )
                un_ap = unc[_P * base : _P * (base + F)].rearrange(
                    "(p f) -> p f", p=_P
                )
                base += F
                pt = io.tile([_P, 2 * F], f32, tag="probs")
                nc.sync.dma_start(out=pt, in_=pr_ap)
                lt = io.tile([_P, F], i32, tag="labs")
                nc.sync.dma_start(out=lt, in_=lb_ap)
                ut = io.tile([_P, F], f32, tag="unc")
                nc.sync.dma_start(out=ut, in_=un_ap)

                p1 = pt[:, 1::2]  # confidences, strided view of interleaved probs

                # tt tile: tanh(unc), later overwritten in place by hm
                tt = mid.tile([_P, F], bf16, tag="tt")
                nc.scalar.activation(tt, ut, Act.Tanh)
                # c2 tile: 2*p1-1, later overwritten by g, ws, aw in place
                c2 = mid.tile([_P, F], bf16, tag="c2")
                nc.scalar.activation(c2, p1, Act.Copy, bias=-1.0, scale=2.0)
                sg = mid.tile([_P, F], bf16, tag="sg")
                nc.scalar.activation(sg, p1, Act.Sign, bias=neg1, scale=2.0)
                # l2 tile: 2*lab-1, overwritten in place by sa
                l2 = mid.tile([_P, F], bf16, tag="l2")
                nc.vector.tensor_scalar(
                    out=l2, in0=lt, scalar1=2.0, scalar2=-1.0,
                    op0=Alu.mult, op1=Alu.add,
                )
                # hm = [unc <= th] - tanh(unc), in place over tt
                nc.vector.scalar_tensor_tensor(
                    tt, ut, float(unc_th), tt, op0=Alu.is_le, op1=Alu.subtract
                )
                # sa = l2 * sg  (= S_a), in place over l2
                nc.vector.tensor_mul(l2, l2, sg)
                # g = sa + c2, in place over c2
                nc.vector.tensor_add(c2, l2, c2)
                # ws = g * hm, in place over c2; fused per-partition sum
                nc.vector.scalar_tensor_tensor(
                    c2, c2, 0.0, tt, op0=Alu.bypass, op1=Alu.mult,
                    accum_out=accA[:, i : i + 1],
                )
                # |ws| on ScalarE, in place; fused per-partition sum
                nc.scalar.activation(
                    c2, c2, Act.Abs, accum_out=absA[:, i : i + 1]
                )
            nc.sync.dma_start(out=out[:, 0:T], in_=accA)
            nc.sync.dma_start(out=out[:, T : 2 * T], in_=absA)
    nc.finalize()  # Bacc: run wait-splitting + register allocation passes
    return nc


def _prep(probs, labels, unc, unc_th):
    probs = np.ascontiguousarray(np.asarray(probs), dtype=np.float32)
    unc = np.ascontiguousarray(np.asarray(unc), dtype=np.float32)
    labels = np.asarray(labels)
    if labels.dtype != np.int32:
        labels = labels.astype(np.int32)  # values are 0/1; lossless narrowing
    labels = np.ascontiguousarray(labels)
    th = float(np.asarray(unc_th))
    assert probs.shape == (_N, 2), probs.shape
    assert unc.shape == (_N,), unc.shape
    assert labels.shape == (_N,), labels.shape

    if th not in _built:
        _built[th] = _build(th)
    nc = _built[th]

    pr = probs.reshape(_NCORES, 2 * _NC)
    lb = labels.reshape(_NCORES, _NC)
    un = unc.reshape(_NCORES, _NC)
    in_maps = [
        {"probs": pr[c], "labs": lb[c], "unc": un[c]} for c in range(_NCORES)
    ]
    return nc, in_maps


def _finish(results):
    S_ws = 0.0
    S_abs = 0.0
    for r in results:
        o = r["out"].astype(np.float64)
        half = o.shape[1] // 2
        S_ws += o[:, :half].sum()
        S_abs += o[:, half:].sum()
    den = S_abs / 2.0
    num = (S_abs + S_ws) / 4.0
    avu = num / (den + 1e-10)
    loss = -1.0 * np.log(avu + 1e-10)
    return np.asarray([loss], dtype=np.float32)


def _run(probs, labels, unc, unc_th, trace=False, **kwargs):
    from concourse.bass_utils import run_bass_kernel_spmd

    nc, in_maps = _prep(probs, labels, unc, unc_th)
    res = run_bass_kernel_spmd(
        nc, in_maps, core_ids=list(range(_NCORES)), trace=trace, **kwargs
    )
    return _finish(res.results), res


def kernel(probs, labels, unc, unc_th):
    out, _ = _run(probs, labels, unc, unc_th, trace=False)
    return out
